# revision 17
# baseline (speedup 1.0000x reference)
"""DN4 episodic kNN scoring kernel for Trainium2 (Bass/Tile).

Per episode t (one NeuronCore each):
  q:(75,640,100) s:(25,640,100) fp32
  qn = q / ||q||_hw (per (wq,c));  sn = s / ||s||_c (per support position)
  rel[wq,way] = qn[wq]^T @ sn[way]  (100x500)
  score[wq,way] = sum over 100 rows of (sum of top-3 of each row's 500)
Output per core: (375,) fp32 = scores in (wq, way) order.

Layout: query rows are packed (wq,hw) -> 7500 columns, processed in 59
chunks of 128 rows (m=128 matmuls). Per-row top-3 sums are regrouped to
per-wq scores with per-chunk indicator matmuls.

Schedule: PE runs kc-outer/way-inner rel matmuls (stationary reuse, 8 PSUM
banks); DVE does MAX8 straight out of PSUM fp32 (no psum->sbuf copy at
all); query-norm prep is cut into per-kc pieces drained one per chunk so
no engine sees a burst. ACT: squares+sqrt (one act table); Pool: scale
writes of normalized Q16.
"""

from collections import deque

import numpy as np

import concourse.bass as bass
import concourse.mybir as mybir
from concourse import bacc
from concourse.tile import TileContext
from concourse.bass_utils import run_bass_kernel_spmd

T, WQ, C, HW = 8, 75, 640, 100
WAY, SHOT, NK = 5, 5, 3
SP = SHOT * HW          # 500 support positions per way
NSP = WAY * SP          # 2500 total support positions
KC = C // 128           # 5 contraction chunks
NPAIR = WQ * WAY        # 375 output scores per episode
NROW = WQ * HW          # 7500 packed query rows
NCH = (NROW + 127) // 128   # 59 row chunks

MINI = 10               # startup query group (covers chunks [0,7))
GRPS = [(10, 20), (30, 25), (55, 20)]   # (wq0, nw) big groups
GMAX = 25

f32 = mybir.dt.float32
f16 = mybir.dt.float16
f8 = mybir.dt.float8e4
AF = mybir.ActivationFunctionType
AX = mybir.AxisListType
OP = mybir.AluOpType
DR = mybir.MatmulPerfMode.DoubleRow

# fp8 DoubleRow layout: kc pairs {0,1},{2,3} in [128, 2, free] tiles (middle
# dim = k-subtile); kc4 stays f16. Strides padded to 16-multiples.
Q8W = 7504              # padded packed-row stride (>= NROW, %16 == 0)
S8WAY = 512             # padded per-way stride (>= SP, %16 == 0)
S8W = WAY * S8WAY


def build_kernel(nc, tc, q_dram, s_dram, ind_dram, out_dram):
    from contextlib import ExitStack

    ctx = ExitStack()
    with ctx:
        const = ctx.enter_context(tc.tile_pool(name="const", bufs=1))
        sn_pool = ctx.enter_context(tc.tile_pool(name="sn", bufs=1))
        q16_pool = ctx.enter_context(tc.tile_pool(name="q16", bufs=1))
        misc = ctx.enter_context(tc.tile_pool(name="misc", bufs=1))
        qlde = ctx.enter_context(tc.tile_pool(name="qlde", bufs=2))
        qsqp = ctx.enter_context(tc.tile_pool(name="qsq", bufs=2))
        qst = ctx.enter_context(tc.tile_pool(name="qst", bufs=4))

        # ---- constants ----
        ones_k = const.tile([128, 1], f16, tag="ones_k")    # partition-reduce lhsT
        nc.vector.memset(ones_k[:], 1.0)
        ones_m = const.tile([1, 128], f16, tag="ones_m")    # broadcast lhsT
        nc.vector.memset(ones_m[:], 1.0)

        # persistent normalized operands: fp8 pairs for kc 0-3, f16 for kc4
        SN8 = [sn_pool.tile([128, 2 * S8W], f8, tag=f"sn8_{p}", name=f"sn8_{p}")
               for p in range(2)]
        SN4 = sn_pool.tile([128, NSP], f16, tag="sn4", name="sn4")
        Q8 = [q16_pool.tile([128, 2 * Q8W], f8, tag=f"q8_{p}", name=f"q8_{p}")
              for p in range(2)]
        Q16_4 = q16_pool.tile([128, NROW], f16, tag="q16_4", name="q16_4")

        def sn8_ap(kc, j):
            # (128, SP) fp8 view of way j in kc's slot
            return SN8[kc // 2][:, (kc % 2) * S8W + j * S8WAY:
                                (kc % 2) * S8W + j * S8WAY + SP]

        def q8_ap(kc, lo, hi):
            return Q8[kc // 2][:, (kc % 2) * Q8W + lo:(kc % 2) * Q8W + hi]

        inv_ns = misc.tile([1, NSP], f16, tag="inv_ns")
        lnv = misc.tile([1, NSP], f32, tag="lnv")
        scores_ch = misc.tile([128, NCH * WAY], f16, tag="scores_ch")
        ind_sb = misc.tile([128, NCH * WQ], f16, tag="ind_sb")

        # ---------- query prep: load + per-kc finish pieces ----------
        def qp_load_kc(pool, width, wq0, nw, kc):
            qg = pool.tile([128, width * HW], f32, tag="qg32", name="qg32")
            src = q_dram[wq0:wq0 + nw, kc * 128:(kc + 1) * 128, :]
            nc.sync.dma_start(
                qg[:, 0:nw * HW].rearrange("c (w h) -> c w h", w=nw),
                src.rearrange("w c h -> c w h"),
            )
            return qg

        def qp_finish_kc(qg, wq0, nw, kc):
            sq = qsqp.tile([128, GMAX * HW], f16, tag="qsq", name="qsq")
            nc.scalar.activation(sq[:, 0:nw * HW], qg[:, 0:nw * HW], AF.Square)
            msq = qst.tile([128, GMAX], f16, tag="msq", name="msq")
            with nc.allow_low_precision("f16 row sumsq (rel tol 2e-2)"):
                nc.vector.tensor_reduce(
                    msq[:, 0:nw],
                    sq[:, 0:nw * HW].rearrange("c (w h) -> c w h", w=nw),
                    axis=AX.X, op=OP.add,
                )
            rmsq = qst.tile([128, GMAX], f32, tag="rmsq", name="rmsq")
            nc.vector.reciprocal(rmsq[:, 0:nw], msq[:, 0:nw])
            rq = qst.tile([128, GMAX], f32, tag="rq", name="rq")
            nc.scalar.sqrt(rq[:, 0:nw], rmsq[:, 0:nw])
            dst = (Q16_4[:, wq0 * HW:(wq0 + nw) * HW] if kc == KC - 1
                   else q8_ap(kc, wq0 * HW, (wq0 + nw) * HW))
            nc.gpsimd.tensor_mul(
                dst.rearrange("c (w h) -> c w h", w=nw),
                qg[:, 0:nw * HW].rearrange("c (w h) -> c w h", w=nw),
                rq[:, 0:nw].to_broadcast([128, nw, HW]),
            )

        # ================= support preparation (single pass) =================
        # s32 kept resident in SBUF: load once, sumsq via ones-matmul, rsqrt
        # via batched Ln/Exp on ACT, broadcast via ones-matmul, scale on DVE.
        sctx = ExitStack()
        sprep = sctx.enter_context(tc.tile_pool(name="sprep", bufs=1))
        sqp = sctx.enter_context(tc.tile_pool(name="sqp", bufs=2))

        S32 = [sprep.tile([128, NSP], f32, tag=f"s32_{kc}", name=f"s32_{kc}")
               for kc in range(KC)]
        for kc in range(KC):
            nc.sync.dma_start(
                S32[kc][:].rearrange("c (w h) -> c w h", w=WAY * SHOT),
                s_dram[:, kc * 128:(kc + 1) * 128, :].rearrange("w c h -> c w h"),
            )
        # prefetch the mini query group DMA right behind s
        mini = [qp_load_kc(qlde, MINI, 0, MINI, kc) for kc in range(KC)]

        with tc.tile_pool(name="ss_psum", bufs=1, space="PSUM") as spsum:
            ss_ps = [spsum.tile([1, SP], f32, tag=f"ss{j}", name=f"ss{j}")
                     for j in range(WAY)]
            for kc in range(KC):
                sq = sqp.tile([128, NSP], f16, tag="sq", name=f"sq_{kc}")
                nc.scalar.activation(sq[:], S32[kc][:], AF.Square)
                for j in range(WAY):
                    nc.tensor.matmul(
                        ss_ps[j][:],
                        ones_k[:],
                        sq[:, j * SP:(j + 1) * SP],
                        start=(kc == 0), stop=(kc == KC - 1),
                    )
            # batched Ln then Exp (avoids per-way act-table swaps)
            for j in range(WAY):
                nc.scalar.activation(
                    lnv[:, j * SP:(j + 1) * SP], ss_ps[j][:], AF.Ln,
                )
            for j in range(WAY):
                nc.scalar.activation(
                    inv_ns[:, j * SP:(j + 1) * SP],
                    lnv[:, j * SP:(j + 1) * SP], AF.Exp, scale=-0.5,
                )
        with tc.tile_pool(name="bc_psum", bufs=1, space="PSUM") as bpsum:
            bc_ps = [bpsum.tile([128, SP], f32, tag=f"bc{j}", name=f"bc{j}")
                     for j in range(WAY)]
            bcs = misc.tile([128, NSP], f16, tag="bcs")
            for j in range(WAY):
                nc.tensor.matmul(
                    bc_ps[j][:], ones_m[:], inv_ns[:, j * SP:(j + 1) * SP],
                    start=True, stop=True,
                )
            for j in range(WAY):
                # GPSIMD can't read PSUM; stage the broadcast norms in SBUF
                nc.scalar.copy(bcs[:, j * SP:(j + 1) * SP], bc_ps[j][:])
            # interleave mini finishes among the scale loops so Q16/SN for
            # the first chunks are ready together; split scales across DVE
            # and GpSimd so neither serializes the startup
            for kc in range(KC):
                eng = nc.gpsimd if kc in (2, 3) else nc.vector
                for j in range(WAY):
                    dst = (SN4[:, j * SP:(j + 1) * SP] if kc == KC - 1
                           else sn8_ap(kc, j))
                    eng.tensor_mul(
                        dst,
                        S32[kc][:, j * SP:(j + 1) * SP],
                        bcs[:, j * SP:(j + 1) * SP],
                    )
                qp_finish_kc(mini[kc], 0, MINI, kc)
        sctx.close()  # free resident s32 SBUF

        # main query pool opens after s32 freed
        qld = ctx.enter_context(tc.tile_pool(name="qld", bufs=6))
        m8buf = ctx.enter_context(tc.tile_pool(name="m8buf", bufs=4))

        # static piece schedule: group g loads at chunk 5g-5 (g1 immediately),
        # finishes drain one per chunk starting at chunk 5(g-1).
        loads = deque()     # (chunk_idx, g)
        fins = deque()      # (chunk_idx, closure)
        pending = {}
        for g, (wq0, nw) in enumerate(GRPS):
            loads.append((5 * g - 3 if g < 2 else 10, g))
            for kc in range(KC):
                # g0 drains 1/chunk (needed by chunk 7); later groups every
                # other chunk to halve the per-chunk DVE insertion
                fins.append((kc if g == 0 else 5 + 10 * (g - 1) + 2 * kc,
                             (g, kc)))

        def chunk_hook(c):
            while loads and loads[0][0] <= c:
                _, g = loads.popleft()
                wq0, nw = GRPS[g]
                for kc in range(KC):
                    pending[(g, kc)] = qp_load_kc(qld, GMAX, wq0, nw, kc)
            while fins and fins[0][0] <= c:
                _, (g, kc) = fins.popleft()
                wq0, nw = GRPS[g]
                qp_finish_kc(pending.pop((g, kc)), wq0, nw, kc)

        with tc.tile_pool(name="rel_psum", bufs=8, space="PSUM") as relp:

            def main_chunk(c):
                c0 = c * 128
                mc = min(128, NROW - c0)
                rel = [relp.tile([128, SP], f32, tag="rel", name=f"rel{c}_{w}")
                       for w in range(WAY)]
                for p in range(2):
                    lhsT = (Q8[p][:].rearrange("c (k n) -> c k n", k=2)
                            [:, :, c0:c0 + mc])
                    rhs3 = SN8[p][:].rearrange(
                        "c (k w n) -> c k w n", k=2, w=WAY)
                    for way in range(WAY):
                        nc.tensor.matmul(
                            rel[way][:mc, :],
                            lhsT,
                            rhs3[:, :, way, 0:SP],
                            start=(p == 0), stop=False,
                            perf_mode=DR,
                        )
                for way in range(WAY):
                    nc.tensor.matmul(
                        rel[way][:mc, :],
                        Q16_4[:, c0:c0 + mc],
                        SN4[:, way * SP:(way + 1) * SP],
                        start=False, stop=True,
                    )
                m8 = m8buf.tile([128, WAY * 8], f32, tag="m8", name="m8")
                for way in range(WAY):
                    nc.vector.max(
                        out=m8[:mc, way * 8:(way + 1) * 8],
                        in_=rel[way][:mc, :],
                    )
                # top-3 sum on GpSimd (keeps DVE free for MAX8)
                m83 = m8[:mc, :].rearrange("p (w e) -> p w e", w=WAY)
                t3 = qst.tile([128, WAY], f32, tag="t3", name="t3")
                with nc.allow_low_precision("f16 per-chunk scores"):
                    nc.gpsimd.tensor_add(t3[:mc, :], m83[:, :, 0], m83[:, :, 1])
                    nc.gpsimd.tensor_add(
                        scores_ch[:mc, c * WAY:(c + 1) * WAY],
                        t3[:mc, :], m83[:, :, 2],
                    )

            # load group 1 before the first chunk
            chunk_hook(-3)
            for c in range(NCH):
                main_chunk(c)
                chunk_hook(c)
                if c == 16:
                    # indicator matrix for the final regroup; issued here so
                    # the DMA lands well before the tail
                    nc.sync.dma_start(
                        ind_sb[:].rearrange("p (n w) -> p n w", n=NCH),
                        ind_dram.rearrange("n p w -> p n w"),
                    )

        # ================= final per-wq regroup =================
        with tc.tile_pool(name="fin_psum", bufs=1, space="PSUM") as finp:
            fin = finp.tile([WQ, WAY], f32, tag="fin")
            for c in range(NCH):
                mc = min(128, NROW - c * 128)
                nc.tensor.matmul(
                    fin[:],
                    ind_sb[:mc, c * WQ:(c + 1) * WQ],
                    scores_ch[:mc, c * WAY:(c + 1) * WAY],
                    start=(c == 0), stop=(c == NCH - 1),
                )
            out_sb = misc.tile([WQ, WAY], f32, tag="out_sb")
            nc.scalar.copy(out_sb[:], fin[:])
            nc.sync.dma_start(out_dram.rearrange("(a b) -> a b", a=WQ), out_sb[:])


_CACHED = {}


def _make_ind():
    ind = np.zeros((NCH, 128, WQ), dtype=np.float16)
    rows = np.arange(NROW)
    for c in range(NCH):
        sel = rows[(rows >= c * 128) & (rows < (c + 1) * 128)]
        ind[c, sel - c * 128, sel // HW] = 1.0
    return ind


def _get_compiled():
    if "nc" in _CACHED:
        return _CACHED["nc"]
    nc = bacc.Bacc(
        "TRN2", target_bir_lowering=False, debug=False,
        enable_asserts=False, num_devices=T,
    )
    q_dram = nc.dram_tensor("q", [WQ, C, HW], f32, kind="ExternalInput").ap()
    s_dram = nc.dram_tensor("s", [WAY * SHOT, C, HW], f32, kind="ExternalInput").ap()
    ind_dram = nc.dram_tensor("ind", [NCH, 128, WQ], f16, kind="ExternalInput").ap()
    out_dram = nc.dram_tensor("out", [NPAIR], f32, kind="ExternalOutput").ap()
    with TileContext(nc) as tc:
        build_kernel(nc, tc, q_dram, s_dram, ind_dram, out_dram)
    nc.compile()
    _CACHED["nc"] = nc
    return nc


def _make_in_maps(query_feat, support_feat):
    q = np.ascontiguousarray(
        np.asarray(query_feat, dtype=np.float32).reshape(T, WQ, C, HW)
    )
    s = np.ascontiguousarray(
        np.asarray(support_feat, dtype=np.float32).reshape(T, WAY * SHOT, C, HW)
    )
    ind = _make_ind()
    return [{"q": q[i], "s": s[i], "ind": ind} for i in range(T)]


def run(query_feat, support_feat):
    nc = _get_compiled()
    in_maps = _make_in_maps(query_feat, support_feat)
    res = run_bass_kernel_spmd(nc, in_maps, core_ids=list(range(T)))
    out = np.stack(
        [res.results[i]["out"].reshape(WQ, WAY) for i in range(T)], axis=0
    ).astype(np.float32)
    return out, res


def kernel(**inputs):
    out, _ = run(inputs["query_feat"], inputs["support_feat"])
    return out
